# revision 14
# baseline (speedup 1.0000x reference)
"""GravityAE GNN kernel v2 for 8 TRN2 NeuronCores (Bass/Tile).

Math (BN folded into weights on host):
  scale_k = gamma_k/sqrt(var_k+eps); shift_k = beta_k + (b_k-mean_k)*scale_k
  W1p = W1*scale1; W2p = W2*scale2
  dinv[n] = 1/sqrt(in_degree incl self loop)
  agg1[d] = segsum_d( (dinv*x)[src] )          (gather table built on host)
  h   = leaky(dinv[d] * (agg1 @ W1p) + shift1)
  hw2 = dinv * (h @ W2p)
  z   = leaky(dinv[d] * segsum_d(hw2[src]) + shift2)
  out[e] = sigmoid(z[dst,64] - ||z[src,:64] - z[dst,:64]||)

Distribution: dst-window sharding. Edges (+self loops) sorted by
(dst_window, src>=HALF); each 128-node window gets C = C_LO + C_HI
chunks of 128 slots (lo-src slots first) so int16-indexed dma_gather
can fetch from <=32768-row table halves. Segment-sum per window is an
indicator matmul accumulated in PSUM. AllGather (x8) rebuilds hw2 and
z tables (fp16, padded to 256B rows). Decode reuses the same slots:
z[src] gathered from z_full, z[dst] from the local z shard; output is
unpermuted to edge order on host.
"""
import numpy as np

P = 128
EPS = 1e-5
HALF = 32768
GATHER_MODE = "ucode"   # "ucode" (dma_gather) | "merged" | "loop"


# --------------------------------------------------------------------------
# host-side preprocessing
# --------------------------------------------------------------------------
def _wrap_idx(vals):
    """dma_gather index layout: idx i at [i%16, i//16], tiled to 128 parts.
    vals: [NWIN, L] int -> [NWIN, 128, L//16] int16 (per-window blocks)."""
    nw, L = vals.shape
    w = np.zeros((nw, 16, (L + 15) // 16), np.int16)
    w[:, np.arange(L) % 16, np.arange(L) // 16] = vals.astype(np.int16)
    return np.tile(w, (1, 8, 1))


def _build_host_tables(x, edge_index, n_cores):
    import ml_dtypes
    N = x.shape[0]
    E = edge_index.shape[1]
    NW = ((N + P - 1) // P + n_cores - 1) // n_cores * n_cores
    NP = NW * P
    NWc = NW // n_cores
    Nc = NWc * P

    src = edge_index[0].astype(np.int64)
    dst = edge_index[1].astype(np.int64)
    # degree includes the self loop; but self loops are NOT in the slot
    # structure (their contribution is added from resident local data)
    deg = np.bincount(np.concatenate([dst, np.arange(N)]),
                      minlength=NP).astype(np.float64)
    dinv = np.zeros(NP, np.float32)
    nz = deg > 0
    dinv[nz] = (1.0 / np.sqrt(deg[nz])).astype(np.float32)

    s_all = src
    d_all = dst
    M = s_all.shape[0]
    win_all = d_all // P
    is_hi = (s_all >= HALF).astype(np.int64)
    order = np.argsort(win_all * 2 + is_hi, kind="stable")
    s_sorted = s_all[order]
    d_sorted = d_all[order]
    w_sorted = win_all[order]
    hi_sorted = s_sorted >= HALF

    counts = np.bincount(w_sorted, minlength=NW)
    lo_counts = np.bincount(w_sorted[~hi_sorted], minlength=NW)
    hi_counts = counts - lo_counts
    C_LO = max(1, int(np.ceil(lo_counts.max() / P)))
    C_HI = max(1, int(np.ceil(hi_counts.max() / P)))
    C = C_LO + C_HI

    starts = np.zeros(NW + 1, np.int64)
    np.cumsum(counts, out=starts[1:])
    k = np.arange(M) - starts[w_sorted]              # rank within window
    slot = np.where(k < lo_counts[w_sorted], k, C_LO * P + k - lo_counts[w_sorted])

    # per-window slot tables
    src_slot = np.zeros((NW, C * P), np.int64)       # pad -> row 0 of its half
    dstf_slot = np.full((NW, C * P), -1.0, np.float32)
    src_half = np.where(hi_sorted, s_sorted - HALF, s_sorted)
    src_slot[w_sorted, slot] = src_half
    dstf_slot[w_sorted, slot] = (d_sorted - w_sorted * P).astype(np.float32)

    # dma_gather index tables (per window, wrapped)
    ixl = _wrap_idx(src_slot[:, : C_LO * P])                   # [NW,128,C_LO*8]
    ixh = _wrap_idx(src_slot[:, C_LO * P:])                    # [NW,128,C_HI*8]

    # global src per slot (pad -> NP-1, an all-zero row)
    offs_slot = src_slot + np.where(
        np.arange(C * P)[None, :] >= C_LO * P, HALF, 0)
    offs_slot = np.where(dstf_slot < 0, NP - 1, offs_slot)

    def to_core_pc(a, width):   # [NW, C*P] -> [n_cores, P, NWc*width] (slot=c*128+p)
        b = a.reshape(n_cores, NWc, width, P)
        return np.ascontiguousarray(b.transpose(0, 3, 1, 2).reshape(n_cores, P, NWc * width))

    # per-partition-replicated dstf rows for the decode S_T build
    dstf_rep = np.broadcast_to(
        dstf_slot.astype(np.float16).reshape(n_cores, 1, NWc * C * P),
        (n_cores, P, NWc * C * P))

    tables = dict(
        dstf=to_core_pc(dstf_slot.astype(np.float16), C),
        dstfrep=np.ascontiguousarray(dstf_rep),
        ixl=np.ascontiguousarray(
            ixl.reshape(n_cores, NWc, 128, C_LO * 8).transpose(0, 2, 1, 3)
            .reshape(n_cores, 128, NWc * C_LO * 8)),
        ixh=np.ascontiguousarray(
            ixh.reshape(n_cores, NWc, 128, C_HI * 8).transpose(0, 2, 1, 3)
            .reshape(n_cores, 128, NWc * C_HI * 8)),
    )
    dinvw = np.ascontiguousarray(
        dinv.reshape(n_cores, NWc, P).transpose(0, 2, 1))      # [cores, P, NWc]

    # host unpermute info: value of sorted edge t lives at
    # outd[core][p, w_local*C + col]
    core_t = w_sorted // NWc
    wl_t = w_sorted % NWc
    col_t = slot // P
    p_t = slot % P
    flat = core_t * (P * NWc * C) + p_t * (NWc * C) + wl_t * C + col_t
    keep = order < E
    out_src_flat = flat[keep]
    out_dst_pos = order[keep]

    return dict(N=N, E=E, NW=NW, NP=NP, NWc=NWc, Nc=Nc,
                C_LO=C_LO, C_HI=C_HI, C=C,
                dinv=dinv, dinvw=dinvw, tables=tables,
                offs_slot=offs_slot,
                out_src_flat=out_src_flat, out_dst_pos=out_dst_pos)


# --------------------------------------------------------------------------
# bass program
# --------------------------------------------------------------------------
def _build_program(NP, NWc, C_LO, C_HI, F1, F2, n_cores):
    import concourse.bass as bass
    import concourse.tile as tile
    from concourse import bacc, mybir, library_config

    dt = mybir.dt
    f32 = dt.float32
    f16 = dt.float16
    i32 = dt.int32
    i16 = dt.int16
    C = C_LO + C_HI
    Nc = NWc * P
    FP = 128                  # padded row width for hw2/z tables
    Fp = F2 - 1               # position dims (64)
    NHI = NP - HALF

    nc = bacc.Bacc("TRN2", target_bir_lowering=False, debug=False,
                   num_devices=n_cores)
    msgt_in = nc.declare_dram_parameter("msgt", [P, NWc * C * F1], f16, isOutput=False)
    w1_in = nc.declare_dram_parameter("w1", [F1, F1], f16, isOutput=False)
    w2_in = nc.declare_dram_parameter("w2", [F1, F2], f16, isOutput=False)
    sd1_in = nc.declare_dram_parameter("shdiv1", [P, NWc * F1], f16, isOutput=False)
    sd2_in = nc.declare_dram_parameter("shdiv2", [P, NWc * F2], f16, isOutput=False)
    iota_in = nc.declare_dram_parameter("iota", [P, C * P], f16, isOutput=False)
    id_in = nc.declare_dram_parameter("ident", [P, P], f16, isOutput=False)
    dinvw_in = nc.declare_dram_parameter("dinvw", [P, NWc], f32, isOutput=False)
    iotap_in = nc.declare_dram_parameter("iotap", [P, 1], f16, isOutput=False)
    xselfT_in = nc.declare_dram_parameter("xselfT", [F1, NWc * P], f16, isOutput=False)
    dstf_in = nc.declare_dram_parameter("dstf", [P, NWc * C], f16, isOutput=False)
    dstfrep_in = nc.declare_dram_parameter("dstfrep", [P, NWc * C * P], f16, isOutput=False)
    ixl_in = nc.declare_dram_parameter("ixl", [P, NWc * C_LO * 8], i16, isOutput=False)
    ixh_in = nc.declare_dram_parameter("ixh", [P, NWc * C_HI * 8], i16, isOutput=False)
    out_dram = nc.declare_dram_parameter("outd", [P, NWc * C], f32, isOutput=True)

    rg = [list(range(n_cores))]

    with tile.TileContext(nc) as tc:
        with (
            tc.tile_pool(name="const", bufs=1) as cpool,
            tc.tile_pool(name="sbuf", bufs=3) as pool,
            tc.tile_pool(name="dec", bufs=2) as dpool2,
            tc.tile_pool(name="psA", bufs=2, space="PSUM") as psA,
            tc.tile_pool(name="dram", bufs=1, space="DRAM") as dpool,
        ):
            nc.gpsimd.load_library(library_config.mlp)
            w1_t = cpool.tile([F1, F1], f16)
            w2_t = cpool.tile([F1, F2], f16)
            sd1_t = cpool.tile([P, NWc * F1], f16)
            sd2_t = cpool.tile([P, NWc * F2], f16)
            iota_t = cpool.tile([P, C * P], f16)
            id_t = cpool.tile([P, P], f16)
            dinvw_t = cpool.tile([P, NWc], f32)
            iotap_t = cpool.tile([P, 1], f16)
            xselfT_t = cpool.tile([F1, NWc * P], f16)
            dstf_t = cpool.tile([P, NWc * C], f16)
            nc.sync.dma_start(out=w1_t[:], in_=w1_in[:])
            nc.sync.dma_start(out=w2_t[:], in_=w2_in[:])
            nc.sync.dma_start(out=sd1_t[:], in_=sd1_in[:])
            nc.sync.dma_start(out=sd2_t[:], in_=sd2_in[:])
            nc.sync.dma_start(out=iota_t[:], in_=iota_in[:])
            nc.sync.dma_start(out=id_t[:], in_=id_in[:])
            nc.sync.dma_start(out=dinvw_t[:], in_=dinvw_in[:])
            nc.sync.dma_start(out=iotap_t[:], in_=iotap_in[:])
            nc.sync.dma_start(out=xselfT_t[:], in_=xselfT_in[:])
            nc.sync.dma_start(out=dstf_t[:], in_=dstf_in[:])
            ixl_t = cpool.tile([P, NWc * C_LO * 8], i16)
            ixh_t = cpool.tile([P, NWc * C_HI * 8], i16)
            nc.sync.dma_start(out=ixl_t[:], in_=ixl_in[:])
            nc.sync.dma_start(out=ixh_t[:], in_=ixh_in[:])
            zloc_t = cpool.tile([P, NWc * F2], f16)
            hw2loc_t = cpool.tile([P, NWc * F2], f16)

            ag2_in = dpool.tile([Nc, FP], f16)
            hw2_full = dpool.tile([NP, FP], f16, addr_space="Shared")
            ag3_in = dpool.tile([Nc, FP], f16)
            z_full = dpool.tile([NP, FP], f16, addr_space="Shared")

            def gather_src(msg, table_lo, table_hi, w, width, tag):
                """Fill msg[P, C, width] with table[src] rows for window w."""
                nc.gpsimd.dma_gather(
                    msg[:, 0:C_LO, :], table_lo,
                    ixl_t[:, w * C_LO * 8:(w + 1) * C_LO * 8],
                    C_LO * P, C_LO * P, width, single_packet=False)
                nc.gpsimd.dma_gather(
                    msg[:, C_LO:C, :], table_hi,
                    ixh_t[:, w * C_HI * 8:(w + 1) * C_HI * 8],
                    C_HI * P, C_HI * P, width, single_packet=False)

            def build_S(w, tag, eng=None):
                # is_equal is symmetric; contiguous iota on in0 (broadcast
                # reads on in0 throttle DVE)
                S_t = pool.tile([P, C, P], f16, tag=tag)
                (eng or nc.vector).tensor_tensor(
                    out=S_t[:],
                    in0=iota_t[:].rearrange("p (c m) -> p c m", m=P),
                    in1=dstf_t[:, w * C:(w + 1) * C]
                        .rearrange("p (c o) -> p c o", o=1).to_broadcast([P, C, P]),
                    op=mybir.AluOpType.is_equal)
                return S_t

            # ---- stage B: layer-1 aggregation + h + hw2, per window ----
            sB = nc.enter_named_scope("aggB", notify=True)
            for w in range(NWc):
                msg1 = pool.tile([P, C, F1], f16, tag="m1")
                nc.sync.dma_start(out=msg1[:],
                                  in_=msgt_in[:, w * C * F1:(w + 1) * C * F1])
                S_t = build_S(w, "S1")
                # aggT[f, m] = sum_e msg1[e, f] * S[e, m]
                ps_aT = psA.tile([F1, P], f32, tag="mmA")
                for c in range(C):
                    nc.tensor.matmul(ps_aT[:], msg1[:, c, :], S_t[:, c, :],
                                     start=(c == 0), stop=(c == C - 1))
                # add the self-loop term (dinv^2 * x)^T while evicting PSUM
                aT_t = pool.tile([F1, P], f16, tag="aT")
                nc.vector.tensor_tensor(
                    out=aT_t[:], in0=ps_aT[:],
                    in1=xselfT_t[:, w * P:(w + 1) * P],
                    op=mybir.AluOpType.add)
                # preload PSUM with shift1/dinv, accumulate aggx @ W1p on top,
                # then h = Lrelu(ps * dinv) in one scalar-engine op
                ps_h = psA.tile([P, F1], f32, tag="mmA")
                nc.scalar.activation(ps_h[:], sd1_t[:, w * F1:(w + 1) * F1],
                                     mybir.ActivationFunctionType.Copy)
                nc.tensor.matmul(ps_h[:], aT_t[:], w1_t[:], start=False, stop=True,
                                 skip_group_check=True)
                s1 = pool.tile([P, F1], f32, tag="s1")
                nc.scalar.activation(s1[:], ps_h[:],
                                     mybir.ActivationFunctionType.Copy,
                                     scale=dinvw_t[:, w:w + 1])
                u1 = pool.tile([P, F1], f32, tag="u1")
                nc.scalar.activation(u1[:], s1[:],
                                     mybir.ActivationFunctionType.Copy, scale=0.1)
                h_t = pool.tile([P, F1], f16, tag="h")
                nc.vector.tensor_tensor(out=h_t[:], in0=s1[:], in1=u1[:],
                                        op=mybir.AluOpType.max)
                ps_tr = psA.tile([P, P], f16, tag="tr")
                nc.tensor.transpose(ps_tr[:], h_t[:], id_t[:])
                hT_t = pool.tile([P, P], f16, tag="hT")
                nc.scalar.activation(hT_t[:], ps_tr[:],
                                     mybir.ActivationFunctionType.Copy)
                ps_w2 = psA.tile([P, F2], f32, tag="mmB")
                nc.tensor.matmul(ps_w2[:], hT_t[:], w2_t[:], start=True, stop=True)
                hw2_t = pool.tile([P, FP], f16, tag="hw2")
                nc.gpsimd.memset(hw2_t[:, F2:FP], 0.0)
                nc.scalar.activation(hw2_t[:, 0:F2], ps_w2[:],
                                     mybir.ActivationFunctionType.Copy,
                                     scale=dinvw_t[:, w:w + 1])
                nc.scalar.activation(hw2loc_t[:, w * F2:(w + 1) * F2],
                                     hw2_t[:, 0:F2],
                                     mybir.ActivationFunctionType.Copy)
                nc.sync.dma_start(out=ag2_in[w * P:(w + 1) * P, :], in_=hw2_t[:])
            nc.leave_named_scope("aggB", sB[0], notify=True)

            sG2 = nc.enter_named_scope("AG2", notify=True)
            nc.gpsimd.collective_compute(
                "AllGather", mybir.AluOpType.bypass,
                ins=[ag2_in.opt()], outs=[hw2_full.opt()], replica_groups=rg)
            nc.leave_named_scope("AG2", sG2[0], notify=True)

            # ---- stage C: layer-2 aggregation + z ----
            sC = nc.enter_named_scope("aggC", notify=True)
            for w in range(NWc):
                msg2 = pool.tile([P, C, FP], f16, tag="m2")
                gather_src(msg2, hw2_full[0:HALF, :], hw2_full[HALF:NP, :],
                           w, FP, "m2")
                S_t = build_S(w, "S2")
                # preload: self-loop message hw2[d] + shift2/dinv, accumulate
                # the edge messages on top, then z = Lrelu(ps * dinv)
                ps_z = psA.tile([P, F2], f32, tag="mmB")
                nc.vector.tensor_tensor(
                    out=ps_z[:], in0=hw2loc_t[:, w * F2:(w + 1) * F2],
                    in1=sd2_t[:, w * F2:(w + 1) * F2], op=mybir.AluOpType.add)
                for c in range(C):
                    nc.tensor.matmul(ps_z[:], S_t[:, c, :], msg2[:, c, 0:F2],
                                     start=False, stop=(c == C - 1),
                                     skip_group_check=True)
                s2 = pool.tile([P, F2], f32, tag="s2")
                nc.scalar.activation(s2[:], ps_z[:],
                                     mybir.ActivationFunctionType.Copy,
                                     scale=dinvw_t[:, w:w + 1])
                u2 = pool.tile([P, F2], f32, tag="u2")
                nc.scalar.activation(u2[:], s2[:],
                                     mybir.ActivationFunctionType.Copy, scale=0.1)
                z_t = pool.tile([P, FP], f16, tag="z")
                nc.vector.memset(z_t[:, F2:FP], 0.0)
                nc.vector.tensor_tensor(out=z_t[:, 0:F2], in0=s2[:], in1=u2[:],
                                        op=mybir.AluOpType.max)
                nc.scalar.activation(zloc_t[:, w * F2:(w + 1) * F2], z_t[:, 0:F2],
                                     mybir.ActivationFunctionType.Copy)
                nc.sync.dma_start(out=ag3_in[w * P:(w + 1) * P, :], in_=z_t[:])
            nc.leave_named_scope("aggC", sC[0], notify=True)

            sG3 = nc.enter_named_scope("AG3", notify=True)
            nc.gpsimd.collective_compute(
                "AllGather", mybir.AluOpType.bypass,
                ins=[ag3_in.opt()], outs=[z_full.opt()], replica_groups=rg)
            nc.leave_named_scope("AG3", sG3[0], notify=True)

            # ---- decode ----
            sD = nc.enter_named_scope("decode", notify=True)
            stage_ss = cpool.tile([P, NWc * C], f32)
            stage_mj = cpool.tile([P, NWc * C], f32)
            for w in range(NWc):
                zs_t = dpool2.tile([P, C, FP], f16, tag="zs")
                gather_src(zs_t, z_full[0:HALF, :], z_full[HALF:NP, :],
                           w, FP, "zs")
                # transposed indicator S_T[m, slot] built from replicated dstf
                dr_t = dpool2.tile([P, C * P], f16, tag="dr")
                nc.sync.dma_start(out=dr_t[:],
                                  in_=dstfrep_in[:, w * C * P:(w + 1) * C * P])
                ST_t = dpool2.tile([P, C * P], f16, tag="ST")
                nc.vector.tensor_tensor(
                    out=ST_t[:],
                    in0=iotap_t[:, 0:1].to_broadcast([P, C * P]),
                    in1=dr_t[:], op=mybir.AluOpType.is_equal)
                # zd[e,:] = sum_m S[e,m] z_w[m,:]; evict via scalar engine
                zd_t = dpool2.tile([P, C, F2], f16, tag="zd")
                for c in range(C):
                    ps_e = psA.tile([P, F2], f32, tag="mmB")
                    nc.tensor.matmul(ps_e[:], ST_t[:, c * P:(c + 1) * P],
                                     zloc_t[:, w * F2:(w + 1) * F2],
                                     start=True, stop=True)
                    nc.scalar.activation(zd_t[:, c, :], ps_e[:],
                                         mybir.ActivationFunctionType.Copy)
                df_t = dpool2.tile([P, C, Fp], f16, tag="df")
                nc.vector.tensor_tensor(out=df_t[:], in0=zs_t[:, :, 0:Fp],
                                        in1=zd_t[:, :, 0:Fp],
                                        op=mybir.AluOpType.subtract)
                nc.vector.tensor_copy(stage_mj[:, w * C:(w + 1) * C],
                                      zd_t[:, :, Fp])
                sq_t = dpool2.tile([P, C, Fp], f32, tag="sq")
                nc.scalar.square(sq_t[:], df_t[:])
                nc.vector.reduce_sum(
                    out=stage_ss[:, w * C:(w + 1) * C]
                        .rearrange("p (c o) -> p c o", o=1),
                    in_=sq_t[:], axis=mybir.AxisListType.X)
            st_d = cpool.tile([P, NWc * C], f32)
            nc.scalar.sqrt(st_d[:], stage_ss[:])
            st_v = cpool.tile([P, NWc * C], f32)
            nc.vector.tensor_tensor(out=st_v[:], in0=stage_mj[:], in1=st_d[:],
                                    op=mybir.AluOpType.subtract)
            st_o = cpool.tile([P, NWc * C], f32)
            nc.scalar.activation(st_o[:], st_v[:],
                                 mybir.ActivationFunctionType.Sigmoid)
            nc.sync.dma_start(out=out_dram[:], in_=st_o[:])
            nc.leave_named_scope("decode", sD[0], notify=True)
    nc.compile()
    return nc


# --------------------------------------------------------------------------
# public entry
# --------------------------------------------------------------------------
def kernel(x, edge_index, W1, b1, gamma1, beta1, mean1, var1,
           W2, b2, gamma2, beta2, mean2, var2, n_cores=8, _trace=False):
    from concourse.bass_utils import run_bass_kernel_spmd

    x = np.asarray(x, np.float32)
    edge_index = np.asarray(edge_index)
    ht = _build_host_tables(x, edge_index, n_cores)
    NP, NWc, Nc = ht["NP"], ht["NWc"], ht["Nc"]
    C_LO, C_HI, C = ht["C_LO"], ht["C_HI"], ht["C"]
    F1 = W1.shape[1]
    F2 = W2.shape[1]

    scale1 = np.asarray(gamma1) / np.sqrt(np.asarray(var1) + EPS)
    shift1 = (np.asarray(beta1) + (np.asarray(b1) - np.asarray(mean1)) * scale1).astype(np.float32)
    W1p = (np.asarray(W1) * scale1[None, :]).astype(np.float16)
    scale2 = np.asarray(gamma2) / np.sqrt(np.asarray(var2) + EPS)
    shift2 = (np.asarray(beta2) + (np.asarray(b2) - np.asarray(mean2)) * scale2).astype(np.float32)
    W2p = (np.asarray(W2) * scale2[None, :]).astype(np.float16)

    xd = np.zeros((NP, F1), np.float16)
    xd[: ht["N"]] = (x * ht["dinv"][: ht["N"], None]).astype(np.float16)
    # host-gathered layer-1 message table, slot layout [p, (w, c, f)]
    msgt = xd[ht["offs_slot"]].reshape(n_cores, NWc, C, P, F1) \
        .transpose(0, 3, 1, 2, 4).reshape(n_cores, P, NWc * C * F1)
    msgt = np.ascontiguousarray(msgt)
    # self-loop message (dinv * x, pre outer-dinv), transposed per window
    xself = np.zeros((NP, F1), np.float16)
    xself[: ht["N"]] = (x * ht["dinv"][: ht["N"], None]).astype(np.float16)
    Nc = ht["Nc"]
    xselfT = np.ascontiguousarray(
        xself.reshape(n_cores, NWc, P, F1).transpose(0, 3, 1, 2)
        .reshape(n_cores, F1, NWc * P))
    iota = np.broadcast_to(
        np.tile(np.arange(P, dtype=np.float16), C)[None, :], (P, C * P)).copy()
    iotap = np.arange(P, dtype=np.float16).reshape(P, 1)
    ident = np.eye(P, dtype=np.float16)
    # shift/dinv PSUM-preload tables (0 where dinv==0, i.e. pad nodes)
    idinv = np.where(ht["dinv"] > 0, 1.0 / np.maximum(ht["dinv"], 1e-30), 0.0)
    sd1 = (idinv[:, None] * shift1[None, :]).astype(np.float16) \
        .reshape(n_cores, NWc, P, F1).transpose(0, 2, 1, 3) \
        .reshape(n_cores, P, NWc * F1)
    sd2 = (idinv[:, None] * shift2[None, :]).astype(np.float16) \
        .reshape(n_cores, NWc, P, F2).transpose(0, 2, 1, 3) \
        .reshape(n_cores, P, NWc * F2)

    tb = ht["tables"]
    in_maps = []
    for c in range(n_cores):
        in_maps.append({
            "msgt": msgt[c], "w1": W1p, "w2": W2p,
            "shdiv1": np.ascontiguousarray(sd1[c]),
            "shdiv2": np.ascontiguousarray(sd2[c]),
            "iota": iota, "iotap": iotap, "ident": ident,
            "dinvw": ht["dinvw"][c],
            "xselfT": xselfT[c],
            "dstf": tb["dstf"][c], "dstfrep": tb["dstfrep"][c],
            "ixl": tb["ixl"][c], "ixh": tb["ixh"][c],
        })
    nc = _build_program(NP, NWc, C_LO, C_HI, F1, F2, n_cores)
    res = run_bass_kernel_spmd(nc, in_maps, list(range(n_cores)), trace=_trace,
                               trace_cores=list(range(n_cores)) if _trace else None)
    outd = np.stack([res.results[c]["outd"] for c in range(n_cores)])  # [cores,P,NWc*C]
    vals = outd.reshape(-1)[ht["out_src_flat"]]
    out = np.empty(ht["E"], np.float32)
    out[ht["out_dst_pos"]] = vals
    kernel._last_results = res
    return out
